# revision 28
# baseline (speedup 1.0000x reference)
"""Trainium2 Bass kernel for nn_L2GESRModule.

Reference computation:
    Fh_conv = Fh @ Wh + bh            (dead: only used via ones_like)
    ESF     = ones_like(Fh_conv)      -> gather indices are a fixed shift
    Y       = Fl @ Wl + bl
    out[b,i,j,:] = Y[b, min(i+1,H-1), min(j+1,W-1), :]

One 1x1-conv GEMM on Fl plus a static (+1,+1) clamped-shift, data-parallel
over batch (1 batch element per core). The Fh/Wh/bh branch is never loaded.

The 2e-2 rel-err budget allows fp16 end-to-end: the host casts Fl/Wl to
fp16 and upcasts the fp16 output, halving HBM traffic (~16.8MB/core ->
~47us at the ~358 GB/s per-core limit). The host also pre-transposes Fl to
FlT [CIN, P+129] (zero-padded) so the kernel needs no PE transposes: X^T
column slices are the stationary matmul operand directly.

Flat-pixel layout: image = 16384 pixels; out[O] = Y[O + 129] except col-127
cells (O%128==127) which need Y[O + 128] (clamped col), and the last row
which duplicates row H-2.

Chunks of CH=128*GK pixels, window W0 = O0+129 (the zero padding keeps the
last chunk's window in bounds, so all chunks are uniform). Group g's
stationary operand is xt[:, kc, :, g] (column j stride GK); psum partition
j then holds pixel W0 + j*GK + g, i.e. ybig[j, g] = Y[O0 + 129 + j*GK + g]
-- GK *consecutive* out pixels per partition -> GK*0.5 KB contiguous per
partition on the store (8/4 KB descriptors; interleaved layouts with 512 B
descriptors measured ~25x slower on the HWDGE store path). The last
chunk's partitions 120-127 compute on padding garbage and are simply not
stored. The clamped edges are pure duplications of interior output
(out[:,:,127]=out[:,:,126] and row 127 = row 126, both exact identities of
the shifted gather), applied on the host after the gather -- the device
stores garbage in those cells and skips the final row entirely.

PSUM evacuation (fp32 psum -> fp16 SBUF, ~4.2M elem/core) runs as one
4-group-wide op per PSUM tile (amortizing the ~120/172-cycle per-op
overhead) split 5:3 between DVE and ACT (the only PSUM-reading engines;
Pool cannot). bl is zero for this module (spec fill=zeros), so the default
build evacuates with plain copies; a with_bias build variant (tensor_add
on DVE) is selected at runtime if bl is ever nonzero.

The PE HAM clock-gate defaults to 4/8 (1.2 GHz) and only reaches 8/8
(2.4 GHz) after ~3.4us of sustained activity: ~24 dependency-free dummy
matmuls run while the first loads are in flight so real matmuls start
warm, and one dummy per chunk resets the idle window whenever a slow load
would otherwise let the PE re-throttle.

Loads go out on the SP HWDGE ring (nc.sync); stores ride SWDGE on the
otherwise-idle Pool/Q7 engine (nc.gpsimd) so neither evacuation engine
ever pauses for a descriptor-generation trigger (on ACT the ~0.65us
trigger sat between evacs and delayed the PSUM pipeline; on SP it
head-of-line blocked load triggers, +9us/core).
"""

import numpy as np

import concourse.bacc as bacc
import concourse.mybir as mybir
from concourse import bass_utils, tile

B, H, W, CIN, COUT = 8, 128, 128, 256, 256
N_CORES = 8
GK = 16                    # pixel-slots per partition per chunk
PW = H * W + 129           # padded FlT width


def build_nc(with_bias: bool = False, n_rows: int = H):
    f16 = mybir.dt.float16
    f32 = mybir.dt.float32
    P = n_rows * W  # total pixels per image
    CH = 128 * GK   # pixels per chunk
    assert P % CH == 0 and P >= 2 * CH
    assert 128 % GK == 0
    n_chunks = P // CH

    nc = bacc.Bacc("TRN2", target_bir_lowering=False, debug=False)
    FlT = nc.dram_tensor("FlT", [CIN, P + 129], f16, kind="ExternalInput").ap()
    Wl = nc.dram_tensor("Wl", [CIN, COUT], f16, kind="ExternalInput").ap()
    if with_bias:
        blb = nc.dram_tensor("blb", [128, COUT], f32, kind="ExternalInput").ap()
    out = nc.dram_tensor("out", [P, COUT], f16, kind="ExternalOutput").ap()

    with tile.TileContext(nc) as tc:
        with (
            tc.tile_pool(name="consts", bufs=1) as consts,
            tc.tile_pool(name="xt", bufs=8) as xt_pool,
            tc.tile_pool(name="yout", bufs=6) as yout_pool,
            tc.tile_pool(name="py", bufs=3, space="PSUM") as py_pool,
            tc.tile_pool(name="warm", bufs=1, space="PSUM") as warm_pool,
        ):
            # Wl as two K-chunks: w_sb[c, kc, n] = Wl[kc*128 + c, n]
            w_sb = consts.tile([128, 2, COUT], f16)
            nc.sync.dma_start(w_sb, Wl.rearrange("(kc kp) n -> kp kc n", kp=128))
            if with_bias:
                bias_sb = consts.tile([128, COUT], f32)
                nc.sync.dma_start(bias_sb, blb)
            # HAM warmup: ~3.5us of dummy matmuls while the first loads are
            # in flight, so the PE clock-gate is at 8/8 (2.4 GHz) when real
            # matmuls start instead of spending the first ~3.4us of them at
            # half rate (measured 13-16us throttle_active without this).
            wrm = consts.tile([128, COUT], f16)
            nc.gpsimd.memset(wrm, 0.0)
            wpy = warm_pool.tile([128, 2, COUT], f32)
            for i in range(16):
                # alternate psum regions so the dummies pipeline back-to-back
                # (a single WAW-chained target leaves ~50% duty and the HAM
                # activity window never fills)
                nc.tensor.matmul(wpy[:, i % 2], wrm[:, 0:128], wrm, start=True, stop=True)

            def evac(q, dst, py, on_dve):
                """dst = py (+ bias) for GE groups at once (amortizes the
                ~120/172-cycle per-op overhead), alternating DVE/ACT so
                neither bottlenecks (Pool cannot read PSUM)."""
                if with_bias:
                    for i in range(GE):
                        nc.vector.tensor_add(dst[:, i], py[:, i], bias_sb)
                elif on_dve:
                    nc.vector.tensor_copy(dst, py)
                else:
                    nc.scalar.copy(dst, py)

            GE = 4  # groups per PSUM tile / evac instruction (2 banks)
            for c in range(n_chunks):
                O0 = CH * c
                W0 = O0 + 129
                # last chunk: partitions 120+ compute on padding, not stored
                nps = (CH - 128) // GK if c == n_chunks - 1 else 128
                xt = xt_pool.tile([128, 2, 128, GK], f16, tag="xt")
                src_w = FlT[:, W0 : W0 + CH].rearrange(
                    "(kc p) (j g) -> p kc j g", p=128, g=GK
                )
                if c == 0:
                    # split the first load so compute ramps earlier
                    nc.sync.dma_start(xt[:, 0], src_w[:, 0])
                    nc.sync.dma_start(xt[:, 1], src_w[:, 1])
                else:
                    nc.sync.dma_start(xt, src_w)
                ybig = yout_pool.tile([128, GK, COUT], f16, tag="yout")
                dst_w = out[O0 : O0 + nps * GK].rearrange("(p k) c -> p k c", k=GK)
                if c > 0:
                    # keep-warm: a dependency-free dummy fires the moment the
                    # PE queue stalls on a late load, resetting HAM's ~3.4us
                    # idle window so the real matmuls stay at 2.4 GHz even
                    # when DMA runs slow (slow cores re-throttle otherwise)
                    nc.tensor.matmul(wpy[:, c % 2], wrm[:, 0:128], wrm, start=True, stop=True)
                for q in range(GK // GE):
                    py = py_pool.tile([128, GE, COUT], f32, tag="py")
                    for gg in range(GE):
                        g = q * GE + gg
                        nc.tensor.matmul(py[:, gg], xt[:, 0, :, g], w_sb[:, 0], start=True, stop=False)
                        nc.tensor.matmul(py[:, gg], xt[:, 1, :, g], w_sb[:, 1], start=False, stop=True)
                    # strict alternation: per-chunk serial evac time stays
                    # ~2.2us/engine, under the PE's ~3.5us/chunk, so the evac
                    # chain never paces the pipeline (a 3-on-DVE split did,
                    # at 3.6us serial -> 4.5us/chunk)
                    evac(q, ybig[:, q * GE : (q + 1) * GE], py, q % 2 == 0)
                    if q == 1:
                        # first half is final once its evacs land: store early
                        # so the write stream overlaps the back-half compute.
                        # Stores ride SWDGE on the otherwise-idle Pool/Q7
                        # engine: on ACT the trigger sits between evacs and
                        # delays the PSUM pipeline ~0.5us/chunk; on the SP
                        # ring it head-of-line blocks load triggers
                        # (measured +9us/core)
                        nc.gpsimd.dma_start(dst_w[0:nps, 0 : 2 * GE], ybig[0:nps, 0 : 2 * GE])
                if c == n_chunks - 1:
                    # split the final store so the kernel-ending write
                    # receipt covers 0.24MB instead of 0.48MB
                    nc.gpsimd.dma_start(dst_w[0:nps, 2 * GE : 3 * GE], ybig[0:nps, 2 * GE : 3 * GE])
                    nc.gpsimd.dma_start(dst_w[0:nps, 3 * GE : GK], ybig[0:nps, 3 * GE : GK])
                else:
                    nc.gpsimd.dma_start(dst_w[0:nps, 2 * GE : GK], ybig[0:nps, 2 * GE : GK])
                # col-127 cells and the duplicated final row are fixed up on
                # the host: out[:,:,127] = out[:,:,126] and row 127 = row 126
                # (both hold identical values by the clamped-shift identity)

    nc.compile()
    return nc


_cache: dict = {}


def _get_nc(with_bias: bool = False):
    key = ("bias", with_bias)
    if key not in _cache:
        _cache[key] = build_nc(with_bias)
    return _cache[key]


def make_in_maps(Fl, Wl, bl):
    Fl = np.asarray(Fl, dtype=np.float32)
    bl = np.asarray(bl, dtype=np.float32)
    with_bias = bool(np.any(bl))
    Wl16 = np.ascontiguousarray(np.asarray(Wl).astype(np.float16))
    P = H * W
    in_maps = []
    for b in range(B):
        FlT = np.zeros((CIN, PW), dtype=np.float16)
        FlT[:, :P] = Fl[b].reshape(P, CIN).T
        m = {"FlT": FlT, "Wl": Wl16}
        if with_bias:
            m["blb"] = np.ascontiguousarray(
                np.broadcast_to(bl, (128, COUT)).astype(np.float32)
            )
        in_maps.append(m)
    return with_bias, in_maps


def kernel(Fh, Fl, Wh, bh, Wl, bl):
    with_bias, in_maps = make_in_maps(Fl, Wl, bl)
    nc = _get_nc(with_bias)
    res = bass_utils.run_bass_kernel_spmd(nc, in_maps, core_ids=list(range(N_CORES)))
    return postprocess([res.results[b]["out"] for b in range(B)])


def postprocess(outs):
    outp = np.stack(
        [o.astype(np.float32).reshape(H, W, COUT) for o in outs], axis=0
    )
    # clamped-shift identities: col 127 duplicates col 126, row 127 row 126
    outp[:, :, W - 1] = outp[:, :, W - 2]
    outp[:, H - 1] = outp[:, H - 2]
    return outp
